# revision 1
# baseline (speedup 1.0000x reference)
"""Trainium2 Bass kernel for nn_MetaRL_LightGAT_BiACT (GAT + LayerNorm + MLP).

Strategy (8 NeuronCores, row-sharded):
  - Each core owns 1024 of the 8192 output rows (node dim N).
  - Host precomputes the tiny GAT projection Wh = x @ W_gat.T and the additive
    attention scores s = Wh @ a.T (0.15% of FLOPs), plus weight transposes.
  - adj is cast to int16 on host (values are 0/1) so the 2-byte DMA-transpose
    (xbar) path can load adj^T slabs directly: the kernel works in a
    transposed layout [j_partition, i_free] so the attention aggregation
    matmul (contraction over j) needs NO on-chip PE transposes of the big
    [N,N] intermediates.
  - Per j-superchunk (8 chunks of 128 j's x 1024 i's):
      DVE:   eT = s_j + s_i                (broadcast-AP add)
      ACT:   q  = exp(leaky_relu(eT))      (Lrelu then Exp, same LUT table set)
      DVE/GpSimd: pT = q * adjT            (mask; 0/1 multiply)
      PE:    acc[ib] += pT_chunk^T @ Whaug (Whaug has a ones column so the
                                            softmax denominator accumulates
                                            as column 48 for free)
  - Epilogue: h' = acc[:, :48] / acc[:, 48], LayerNorm (bn_stats/bn_aggr),
    MLP 48->256->128->32 in transposed layout on PE.
"""

import sys

if "/opt/trn_rl_repo" not in sys.path:
    sys.path.insert(0, "/opt/trn_rl_repo")

import numpy as np

N = 8192
D_IN = 128
D_H = 48
D_AUG = 65  # Wh cols 0-47, zeros 48-63, ones col at 64 (quadrant-aligned)
D_OUT = 32
N_CORES = 8
ROWS = N // N_CORES          # 1024 rows per core
P = 128                      # partitions
N_IBLK = ROWS // P           # 8 i-blocks per core
N_CHUNK = N // P             # 64 j-chunks
SC_CHUNKS = 8                # j-chunks per superchunk
N_SC = N_CHUNK // SC_CHUNKS  # 8 superchunks
NEG_SLOPE = 0.2
EPS = 1e-5


def build_nc(num_cores=N_CORES, rows=ROWS, n=N, dve_mask_chunks=8, q_bufs=2,
             adj_bufs=3, lrelu_mode="act", reps=1, add_mode="ts",
             stages="dma,add,act,mask,mm,epi"):
    import concourse.bass as bass
    import concourse.mybir as mybir
    import concourse.tile as tile
    from concourse import bacc
    from concourse.masks import make_identity
    from contextlib import ExitStack

    f32 = mybir.dt.float32
    i16 = mybir.dt.int16
    AF = mybir.ActivationFunctionType
    OP = mybir.AluOpType

    n_iblk = rows // P
    n_chunk = n // P
    n_sc = max(1, n_chunk // SC_CHUNKS)
    sc_chunks = n_chunk // n_sc

    st = {}
    for tok in stages.split(","):
        name, _, mult = tok.partition(":")
        st[name] = int(mult) if mult else 1
    nc = bacc.Bacc("TRN2", target_bir_lowering=False, debug=False,
                   num_devices=num_cores)

    adj16_d = nc.dram_tensor("adj16", [rows, n], i16, kind="ExternalInput").ap()
    whaug_d = nc.dram_tensor("whaug", [n, D_AUG], f32, kind="ExternalInput").ap()
    sP_d = nc.dram_tensor("sP", [P, n_chunk], f32, kind="ExternalInput").ap()
    sI_d = nc.dram_tensor("sI", [1, rows], f32, kind="ExternalInput").ap()
    gamma_d = nc.dram_tensor("gamma", [1, D_H], f32, kind="ExternalInput").ap()
    beta_d = nc.dram_tensor("beta", [1, D_H], f32, kind="ExternalInput").ap()
    w1t_d = nc.dram_tensor("w1t", [D_H, 256], f32, kind="ExternalInput").ap()
    b1_d = nc.dram_tensor("b1", [256, 1], f32, kind="ExternalInput").ap()
    w2t_d = nc.dram_tensor("w2t", [256, 128], f32, kind="ExternalInput").ap()
    b2_d = nc.dram_tensor("b2", [128, 1], f32, kind="ExternalInput").ap()
    w3t_d = nc.dram_tensor("w3t", [128, D_OUT], f32, kind="ExternalInput").ap()
    b3_d = nc.dram_tensor("b3", [D_OUT, 1], f32, kind="ExternalInput").ap()
    out_d = nc.dram_tensor("out", [rows, D_OUT], f32, kind="ExternalOutput").ap()

    with ExitStack() as ctx:
        tc = ctx.enter_context(tile.TileContext(nc))
        singles = ctx.enter_context(tc.tile_pool(name="singles", bufs=1))
        adjp = ctx.enter_context(tc.tile_pool(name="adjp", bufs=adj_bufs))
        qp = ctx.enter_context(tc.tile_pool(name="qp", bufs=q_bufs))
        hp = ctx.enter_context(tc.tile_pool(name="hp", bufs=2))

        # ---- resident small tensors ----
        whaug_sb = singles.tile([P, n_chunk, D_AUG], f32)
        nc.sync.dma_start(whaug_sb, whaug_d.rearrange("(c p) d -> p c d", p=P))
        sP_sb = singles.tile([P, n_chunk], f32)
        nc.sync.dma_start(sP_sb, sP_d)
        sI_sb = singles.tile([P, rows], f32)
        nc.sync.dma_start(sI_sb, sI_d[0:1, :].partition_broadcast(P).rearrange(
            "p one r -> p (one r)"))
        gamma_sb = singles.tile([P, D_H], f32)
        nc.sync.dma_start(gamma_sb, gamma_d[0:1, :].partition_broadcast(P)
                          .rearrange("p one r -> p (one r)"))
        beta_sb = singles.tile([P, D_H], f32)
        nc.sync.dma_start(beta_sb, beta_d[0:1, :].partition_broadcast(P)
                          .rearrange("p one r -> p (one r)"))
        w1t_sb = singles.tile([D_H, 256], f32)
        nc.sync.dma_start(w1t_sb, w1t_d)
        w2t_sb = singles.tile([P, 2, 128], f32)
        nc.sync.dma_start(w2t_sb, w2t_d.rearrange("(m p) k -> p m k", p=P))
        w3t_sb = singles.tile([P, D_OUT], f32)
        nc.sync.dma_start(w3t_sb, w3t_d)
        b1_sb = singles.tile([P, 2], f32)
        nc.sync.dma_start(b1_sb, b1_d.rearrange("(m p) one -> p (m one)", p=P))
        b2_sb = singles.tile([P, 1], f32)
        nc.sync.dma_start(b2_sb, b2_d)
        b3_sb = singles.tile([D_OUT, 1], f32)
        nc.sync.dma_start(b3_sb, b3_d)
        eps_sb = singles.tile([P, 1], f32)
        nc.vector.memset(eps_sb, EPS)
        ident = singles.tile([P, P], f32)
        make_identity(nc, ident)

        def bcast_sb(dst, src_row, parts):
            src = bass.AP(tensor=src_row.tensor, offset=src_row.offset,
                          ap=[src_row.ap[0], [0, parts], src_row.ap[1]])
            dst3 = bass.AP(tensor=dst.tensor, offset=dst.offset,
                           ap=[dst.ap[0], [1, 1], dst.ap[1]])
            nc.sync.dma_start(dst3, src)
        ones48 = singles.tile([D_H, 1], f32)
        nc.vector.memset(ones48, 1.0)
        gammaC = singles.tile([D_H, 1], f32)
        nc.sync.dma_start(gammaC, gamma_d.rearrange("one d -> d one"))
        betaC = singles.tile([D_H, 1], f32)
        nc.sync.dma_start(betaC, beta_d.rearrange("one d -> d one"))

        # ---- main loop: attention aggregation in transposed layout ----
        n_half = rows // 512
        for rep in range(reps):
          with tc.tile_pool(name=f"accp{rep}", bufs=2,
                            space="PSUM") as accp:
            acc = [accp.tile([D_AUG, 512], f32, tag="acc", name=f"acc{i}")
                   for i in range(n_half)]
            for sc in range(n_sc):
                adjT = adjp.tile([P, sc_chunks, rows], i16)
                for _m in range(st.get("dma", 0)):
                    for cc in range(sc_chunks):
                        jc = sc * sc_chunks + cc
                        nc.sync.dma_start(adjT[:, cc, :],
                                          adj16_d[:, jc * P:(jc + 1) * P],
                                          transpose=True)
                q = qp.tile([P, sc_chunks, rows], f32)
                # eT[jp, cc, i] = s_j(jp, sc*8+cc) + s_i(i)
                sj = sP_sb[:, sc * sc_chunks:(sc + 1) * sc_chunks]
                in0 = bass.AP(tensor=sj.tensor, offset=sj.offset,
                              ap=list(sj.ap) + [[0, rows]])
                in1 = bass.AP(tensor=sI_sb.tensor, offset=sI_sb.offset,
                              ap=[sI_sb.ap[0], [0, sc_chunks], sI_sb.ap[1]])
                for _m in range(st.get("add", 0)):
                    if add_mode == "ts":
                        for cc in range(sc_chunks):
                            jc = sc * sc_chunks + cc
                            nc.vector.tensor_scalar(
                                q[:, cc, :], sI_sb, sP_sb[:, jc:jc + 1],
                                None, OP.add)
                    else:
                        nc.vector.tensor_tensor(q, in0, in1, OP.add)
                qf = q.rearrange("p a b -> p (a b)")
                for _m in range(st.get("act", 0)):
                    if lrelu_mode == "act":
                        nc.scalar.activation(qf, qf, AF.Prelu,
                                             alpha=NEG_SLOPE)
                        nc.scalar.activation(qf, qf, AF.Exp)
                    else:  # exp(leaky(x)) == max(exp(x), exp(0.2 x))
                        q2 = qp.tile([P, sc_chunks, rows], f32, name="q2",
                                     tag="q2")
                        q2f = q2.rearrange("p a b -> p (a b)")
                        nc.scalar.activation(q2f, qf, AF.Exp,
                                             scale=NEG_SLOPE)
                        nc.scalar.activation(qf, qf, AF.Exp)
                        nc.vector.tensor_tensor(qf, qf, q2f, OP.max)
                # mask multiply, split DVE / GpSimd
                dm = min(dve_mask_chunks, sc_chunks)
                for _m in range(st.get("mask", 0)):
                    nc.vector.tensor_tensor(q[:, :dm, :], q[:, :dm, :],
                                            adjT[:, :dm, :], OP.mult)
                    if dm < sc_chunks:
                        nc.gpsimd.tensor_tensor(q[:, dm:, :], q[:, dm:, :],
                                                adjT[:, dm:, :], OP.mult)
                n_mm = st.get("mm", 0)
                for _m in range(n_mm):
                    for cc in range(sc_chunks):
                        jc = sc * sc_chunks + cc
                        for h in range(n_half):
                            nc.tensor.matmul(
                                acc[h][:, :],
                                lhsT=whaug_sb[:, jc, :],
                                rhs=q[:, cc, h * 512:(h + 1) * 512],
                                start=(jc == 0 and _m == 0),
                                stop=(jc == n_chunk - 1 and _m == n_mm - 1))

            # ---- epilogue phase 1: h' + LayerNorm (T-layout) -> SBUF ----
            hs = []
            do_epi = st.get("epi", 0) > 0 and st.get("mm", 0) > 0
            for h in range(n_half if do_epi else 0):
                rec = hp.tile([1, 512], f32, tag="rec")
                nc.vector.reciprocal(rec, acc[h][64:65, :])
                rbc = hp.tile([D_H, 512], f32, tag="rbc")
                bcast_sb(rbc, rec[0:1, :], D_H)
                hT = hp.tile([D_H, 512], f32, tag="hT", bufs=n_half)
                nc.vector.tensor_tensor(hT, acc[h][0:D_H, :], rbc, OP.mult)
                sq = hp.tile([D_H, 512], f32, tag="sq")
                nc.scalar.activation(sq, hT, AF.Square)
                ssum = accp.tile([1, 512], f32, tag="ssum", name="ssum")
                nc.tensor.matmul(ssum, lhsT=ones48, rhs=hT,
                                 start=True, stop=True)
                ssq = accp.tile([1, 512], f32, tag="ssq", name="ssq")
                nc.tensor.matmul(ssq, lhsT=ones48, rhs=sq,
                                 start=True, stop=True)
                mean = hp.tile([1, 512], f32, tag="mean")
                nc.scalar.activation(mean, ssum, AF.Copy, scale=1.0 / D_H)
                var = hp.tile([1, 512], f32, tag="var")
                nc.scalar.activation(var, ssq, AF.Copy, scale=1.0 / D_H)
                msq = hp.tile([1, 512], f32, tag="msq")
                nc.vector.tensor_tensor(msq, mean, mean, OP.mult)
                nc.vector.tensor_tensor(var, var, msq, OP.subtract)
                std = hp.tile([1, 512], f32, tag="std")
                nc.scalar.activation(std, var, AF.Sqrt, bias=eps_sb[0:1, :])
                rstd = hp.tile([1, 512], f32, tag="rstd")
                nc.vector.reciprocal(rstd, std)
                mbc = hp.tile([D_H, 512], f32, tag="mbc")
                bcast_sb(mbc, mean[0:1, :], D_H)
                sbc = hp.tile([D_H, 512], f32, tag="sbc")
                bcast_sb(sbc, rstd[0:1, :], D_H)
                nc.vector.tensor_tensor(hT, hT, mbc, OP.subtract)
                nc.vector.tensor_tensor(hT, hT, sbc, OP.mult)
                nc.vector.tensor_scalar(hT, hT, gammaC, betaC,
                                        OP.mult, OP.add)
                hs.append(hT)

          # ---- epilogue phase 2: MLP head in transposed layout ----
          with tc.tile_pool(name=f"mlpp{rep}", bufs=1, space="PSUM") as mlpp:
            for h in range(n_half if do_epi else 0):
                h1 = hp.tile([P, 2, 512], f32, tag="h1")
                for m in range(2):
                    m1 = mlpp.tile([P, 512], f32, tag="m1")
                    nc.tensor.matmul(m1, lhsT=w1t_sb[:, m * P:(m + 1) * P],
                                     rhs=hs[h], start=True, stop=True)
                    nc.scalar.activation(h1[:, m, :], m1, AF.Relu,
                                         bias=b1_sb[:, m:m + 1])
                m2 = mlpp.tile([P, 512], f32, tag="m2")
                for m in range(2):
                    nc.tensor.matmul(m2, lhsT=w2t_sb[:, m, :],
                                     rhs=h1[:, m, :],
                                     start=(m == 0), stop=(m == 1))
                h2 = hp.tile([P, 512], f32, tag="h2")
                nc.scalar.activation(h2, m2, AF.Relu, bias=b2_sb)
                m3 = mlpp.tile([D_OUT, 512], f32, tag="m3")
                nc.tensor.matmul(m3, lhsT=w3t_sb, rhs=h2,
                                 start=True, stop=True)
                h3 = hp.tile([D_OUT, 512], f32, tag="h3")
                nc.scalar.activation(h3, m3, AF.Identity, bias=b3_sb)
                for k in range(4):
                    ko = h * 4 + k
                    m4 = mlpp.tile([P, D_OUT], f32, tag="m4")
                    nc.tensor.transpose(m4, h3[:, k * P:(k + 1) * P],
                                        ident[0:D_OUT, 0:D_OUT])
                    ob = hp.tile([P, D_OUT], f32, tag="ob")
                    nc.vector.tensor_copy(ob, m4)
                    nc.sync.dma_start(out_d[ko * P:(ko + 1) * P, :], ob)

    nc.compile()
    return nc


def host_prep(x, adj, W_gat, a, gamma, beta, W1, b1, W2, b2, W3, b3,
              num_cores=N_CORES):
    n = x.shape[0]
    rows = n // num_cores
    n_chunk = n // P
    Wh = (x @ W_gat.T).astype(np.float32)
    s = (Wh @ a.T).astype(np.float32).ravel()
    whaug = np.concatenate([Wh, np.zeros((n, 17), np.float32)], axis=1)
    whaug[:, 64] = 1.0
    whaug = np.ascontiguousarray(whaug)
    adj16 = adj.astype(np.int16)
    sP = np.ascontiguousarray(s.reshape(n_chunk, P).T)
    in_maps = []
    for c in range(num_cores):
        r = slice(c * rows, (c + 1) * rows)
        in_maps.append({
            "adj16": np.ascontiguousarray(adj16[r]),
            "whaug": whaug,
            "sP": sP,
            "sI": np.ascontiguousarray(s[r][None, :]),
            "gamma": np.ascontiguousarray(gamma[None, :]).astype(np.float32),
            "beta": np.ascontiguousarray(beta[None, :]).astype(np.float32),
            "w1t": np.ascontiguousarray(W1.T).astype(np.float32),
            "b1": np.ascontiguousarray(b1[:, None]).astype(np.float32),
            "w2t": np.ascontiguousarray(W2.T).astype(np.float32),
            "b2": np.ascontiguousarray(b2[:, None]).astype(np.float32),
            "w3t": np.ascontiguousarray(W3.T).astype(np.float32),
            "b3": np.ascontiguousarray(b3[:, None]).astype(np.float32),
        })
    return in_maps


_NC_CACHE = {}


def kernel(x, adj, W_gat, a, gamma, beta, W1, b1, W2, b2, W3, b3,
           trace=False):
    from concourse.bass_utils import run_bass_kernel_spmd

    args = [np.asarray(t) for t in
            (x, adj, W_gat, a, gamma, beta, W1, b1, W2, b2, W3, b3)]
    in_maps = host_prep(*args)
    if "nc" not in _NC_CACHE:
        _NC_CACHE["nc"] = build_nc()
    nc = _NC_CACHE["nc"]
    res = run_bass_kernel_spmd(nc, in_maps, list(range(N_CORES)), trace=trace)
    out = np.concatenate([r["out"] for r in res.results], axis=0)
    if trace:
        kernel.last_results = res
    return out.astype(np.float32)



# revision 2
# speedup vs baseline: 1.5141x; 1.5141x over previous
"""Trainium2 Bass kernel for nn_MetaRL_LightGAT_BiACT (GAT + LayerNorm + MLP).

Strategy (8 NeuronCores, row-sharded, transposed layout [j_part, i_free]):
  - Each core owns 1024 of the 8192 output rows (node dim N=i); the full
    j dim (8192) is reduced on-chip via PSUM accumulation.
  - Host precomputes the tiny GAT projection Wh = x @ W_gat.T and scores
    s = Wh @ a.T (0.15% of FLOPs), and marshals adj into a single bf16
    tensor  adjm[j, i] = adj[i, j] ? s_i : -60   (pre-transposed and
    pre-tiled so each superchunk is one contiguous [128, 8192] DMA slab).
  - Identity used on device, per element (w = adjm):
        adj * exp(prelu(s_i + s_j))
      = exp(max(w, 0.2*w - 0.8*s_j) + s_j)            (w = s_i on edges)
      = exp(max(w, 0.2*w - 0.8*s_j)) * e^{s_j}
    with e^{s_j} folded into the matmul weights WhU[j,:] = e^{s_j}*Wh[j,:]
    (and the softmax-denominator ones column becomes e^{s_j}).
    Non-edges (w = -60) give exp(<= -11) ~ 0, i.e. the mask.
  - Device main loop per superchunk (1024 j's):
      DVE  tensor_scalar (bf16 4x): t1 = 0.2*w - 0.8*s_j   (per 128-chunk)
      DVE  tensor_tensor (bf16 2x): q = max(w, t1)         (whole slab)
      ACT  activation    Exp       : q = exp(q)            (whole slab)
      PE   matmul bf16: acc[65, i] += WhU_chunk^T @ q_chunk  (PSUM accum,
           col 64 of WhU is e^{s_j} -> denominator accumulates for free)
  - Epilogue: h' = acc[:, :48] / acc[:, 64], LayerNorm, MLP 48->256->128->32
    in transposed layout on PE (fp32, small).
"""

import sys

if "/opt/trn_rl_repo" not in sys.path:
    sys.path.insert(0, "/opt/trn_rl_repo")

import numpy as np
import ml_dtypes

N = 8192
D_IN = 128
D_H = 48
D_AUG = 65  # WhU cols 0-47, zeros 48-63, e^{s_j} col at 64
D_OUT = 32
N_CORES = 8
ROWS = N // N_CORES          # 1024 rows per core
P = 128                      # partitions
SC_CHUNKS = 8                # j-chunks per superchunk
MASK_VAL = -60.0
EPS = 1e-5


def build_nc(num_cores=N_CORES, rows=ROWS, n=N, reps=1):
    import concourse.bass as bass
    import concourse.mybir as mybir
    import concourse.tile as tile
    from concourse import bacc
    from concourse.masks import make_identity
    from contextlib import ExitStack

    f32 = mybir.dt.float32
    bf16 = mybir.dt.bfloat16
    AF = mybir.ActivationFunctionType
    OP = mybir.AluOpType

    n_chunk = n // P
    n_sc = max(1, n_chunk // SC_CHUNKS)
    sc_chunks = n_chunk // n_sc
    n_half = rows // 512

    nc = bacc.Bacc("TRN2", target_bir_lowering=False, debug=False,
                   num_devices=num_cores)

    adjm_d = nc.dram_tensor("adjm", [n_sc * P, sc_chunks * rows], bf16,
                            kind="ExternalInput").ap()
    whu_d = nc.dram_tensor("whu", [P, n_chunk * D_AUG], bf16,
                           kind="ExternalInput").ap()
    sJm_d = nc.dram_tensor("sJm", [P, n_chunk], f32, kind="ExternalInput").ap()
    gamma_d = nc.dram_tensor("gamma", [1, D_H], f32, kind="ExternalInput").ap()
    beta_d = nc.dram_tensor("beta", [1, D_H], f32, kind="ExternalInput").ap()
    w1t_d = nc.dram_tensor("w1t", [D_H, 256], f32, kind="ExternalInput").ap()
    b1_d = nc.dram_tensor("b1", [256, 1], f32, kind="ExternalInput").ap()
    w2t_d = nc.dram_tensor("w2t", [256, 128], f32, kind="ExternalInput").ap()
    b2_d = nc.dram_tensor("b2", [128, 1], f32, kind="ExternalInput").ap()
    w3t_d = nc.dram_tensor("w3t", [128, D_OUT], f32, kind="ExternalInput").ap()
    b3_d = nc.dram_tensor("b3", [D_OUT, 1], f32, kind="ExternalInput").ap()
    out_d = nc.dram_tensor("out", [rows, D_OUT], f32, kind="ExternalOutput").ap()

    with ExitStack() as ctx:
        tc = ctx.enter_context(tile.TileContext(nc))
        singles = ctx.enter_context(tc.tile_pool(name="singles", bufs=1))
        adjp = ctx.enter_context(tc.tile_pool(name="adjp", bufs=3))
        t1p = ctx.enter_context(tc.tile_pool(name="t1p", bufs=2))
        qp = ctx.enter_context(tc.tile_pool(name="qp", bufs=2))
        hp = ctx.enter_context(tc.tile_pool(name="hp", bufs=2))

        # ---- resident small tensors ----
        whu_sb = singles.tile([P, n_chunk, D_AUG], bf16)
        nc.sync.dma_start(whu_sb, whu_d.rearrange("p (c d) -> p c d", d=D_AUG))
        sJm_sb = singles.tile([P, n_chunk], f32)
        nc.sync.dma_start(sJm_sb, sJm_d)
        gamma_sb = singles.tile([P, D_H], f32)
        nc.sync.dma_start(gamma_sb, gamma_d[0:1, :].partition_broadcast(P)
                          .rearrange("p one r -> p (one r)"))
        beta_sb = singles.tile([P, D_H], f32)
        nc.sync.dma_start(beta_sb, beta_d[0:1, :].partition_broadcast(P)
                          .rearrange("p one r -> p (one r)"))
        w1t_sb = singles.tile([D_H, 256], f32)
        nc.sync.dma_start(w1t_sb, w1t_d)
        w2t_sb = singles.tile([P, 2, 128], f32)
        nc.sync.dma_start(w2t_sb, w2t_d.rearrange("(m p) k -> p m k", p=P))
        w3t_sb = singles.tile([P, D_OUT], f32)
        nc.sync.dma_start(w3t_sb, w3t_d)
        b1_sb = singles.tile([P, 2], f32)
        nc.sync.dma_start(b1_sb, b1_d.rearrange("(m p) one -> p (m one)", p=P))
        b2_sb = singles.tile([P, 1], f32)
        nc.sync.dma_start(b2_sb, b2_d)
        b3_sb = singles.tile([D_OUT, 1], f32)
        nc.sync.dma_start(b3_sb, b3_d)
        eps_sb = singles.tile([P, 1], f32)
        nc.vector.memset(eps_sb, EPS)
        ident = singles.tile([P, P], f32)
        make_identity(nc, ident)

        def bcast_sb(dst, src_row, parts):
            src = bass.AP(tensor=src_row.tensor, offset=src_row.offset,
                          ap=[src_row.ap[0], [0, parts], src_row.ap[1]])
            dst3 = bass.AP(tensor=dst.tensor, offset=dst.offset,
                           ap=[dst.ap[0], [1, 1], dst.ap[1]])
            nc.sync.dma_start(dst3, src)

        ones48 = singles.tile([D_H, 1], f32)
        nc.vector.memset(ones48, 1.0)
        gammaC = singles.tile([D_H, 1], f32)
        nc.sync.dma_start(gammaC, gamma_d.rearrange("one d -> d one"))
        betaC = singles.tile([D_H, 1], f32)
        nc.sync.dma_start(betaC, beta_d.rearrange("one d -> d one"))

        # ---- main loop: masked attention scores + aggregation ----
        for rep in range(reps):
          with tc.tile_pool(name=f"accp{rep}", bufs=2, space="PSUM") as accp:
            acc = [accp.tile([D_AUG, 512], f32, tag="acc", name=f"acc{i}")
                   for i in range(n_half)]
            for sc in range(n_sc):
                adjm = adjp.tile([P, sc_chunks, rows], bf16)
                nc.sync.dma_start(
                    adjm.rearrange("p a b -> p (a b)"),
                    adjm_d[sc * P:(sc + 1) * P, :])
                t1 = t1p.tile([P, sc_chunks, rows], bf16)
                for cc in range(sc_chunks):
                    jc = sc * sc_chunks + cc
                    nc.vector.tensor_scalar(
                        t1[:, cc, :], adjm[:, cc, :],
                        0.2, sJm_sb[:, jc:jc + 1], OP.mult, OP.add)
                q = qp.tile([P, sc_chunks, rows], bf16)
                qf = q.rearrange("p a b -> p (a b)")
                nc.vector.tensor_tensor(
                    qf, adjm.rearrange("p a b -> p (a b)"),
                    t1.rearrange("p a b -> p (a b)"), OP.max)
                nc.scalar.activation(qf, qf, AF.Exp)
                for cc in range(sc_chunks):
                    jc = sc * sc_chunks + cc
                    for h in range(n_half):
                        nc.tensor.matmul(
                            acc[h][:, :],
                            lhsT=whu_sb[:, jc, :],
                            rhs=q[:, cc, h * 512:(h + 1) * 512],
                            start=(jc == 0),
                            stop=(jc == n_chunk - 1))

            # ---- epilogue phase 1: h' + LayerNorm (T-layout) -> SBUF ----
            hs = []
            for h in range(n_half):
                rec = hp.tile([1, 512], f32, tag="rec")
                nc.vector.reciprocal(rec, acc[h][64:65, :])
                rbc = hp.tile([D_H, 512], f32, tag="rbc")
                bcast_sb(rbc, rec[0:1, :], D_H)
                hT = hp.tile([D_H, 512], f32, tag="hT", bufs=n_half)
                nc.vector.tensor_tensor(hT, acc[h][0:D_H, :], rbc, OP.mult)
                sq = hp.tile([D_H, 512], f32, tag="sq")
                nc.scalar.activation(sq, hT, AF.Square)
                ssum = accp.tile([1, 512], f32, tag="ssum", name="ssum")
                nc.tensor.matmul(ssum, lhsT=ones48, rhs=hT,
                                 start=True, stop=True)
                ssq = accp.tile([1, 512], f32, tag="ssq", name="ssq")
                nc.tensor.matmul(ssq, lhsT=ones48, rhs=sq,
                                 start=True, stop=True)
                mean = hp.tile([1, 512], f32, tag="mean")
                nc.scalar.activation(mean, ssum, AF.Copy, scale=1.0 / D_H)
                var = hp.tile([1, 512], f32, tag="var")
                nc.scalar.activation(var, ssq, AF.Copy, scale=1.0 / D_H)
                msq = hp.tile([1, 512], f32, tag="msq")
                nc.vector.tensor_tensor(msq, mean, mean, OP.mult)
                nc.vector.tensor_tensor(var, var, msq, OP.subtract)
                std = hp.tile([1, 512], f32, tag="std")
                nc.scalar.activation(std, var, AF.Sqrt, bias=eps_sb[0:1, :])
                rstd = hp.tile([1, 512], f32, tag="rstd")
                nc.vector.reciprocal(rstd, std)
                mbc = hp.tile([D_H, 512], f32, tag="mbc")
                bcast_sb(mbc, mean[0:1, :], D_H)
                sbc = hp.tile([D_H, 512], f32, tag="sbc")
                bcast_sb(sbc, rstd[0:1, :], D_H)
                nc.vector.tensor_tensor(hT, hT, mbc, OP.subtract)
                nc.vector.tensor_tensor(hT, hT, sbc, OP.mult)
                nc.vector.tensor_scalar(hT, hT, gammaC, betaC,
                                        OP.mult, OP.add)
                hs.append(hT)

          # ---- epilogue phase 2: MLP head in transposed layout ----
          with tc.tile_pool(name=f"mlpp{rep}", bufs=1, space="PSUM") as mlpp:
            for h in range(n_half):
                h1 = hp.tile([P, 2, 512], f32, tag="h1")
                for m in range(2):
                    m1 = mlpp.tile([P, 512], f32, tag="m1")
                    nc.tensor.matmul(m1, lhsT=w1t_sb[:, m * P:(m + 1) * P],
                                     rhs=hs[h], start=True, stop=True)
                    nc.scalar.activation(h1[:, m, :], m1, AF.Relu,
                                         bias=b1_sb[:, m:m + 1])
                m2 = mlpp.tile([P, 512], f32, tag="m2")
                for m in range(2):
                    nc.tensor.matmul(m2, lhsT=w2t_sb[:, m, :],
                                     rhs=h1[:, m, :],
                                     start=(m == 0), stop=(m == 1))
                h2 = hp.tile([P, 512], f32, tag="h2")
                nc.scalar.activation(h2, m2, AF.Relu, bias=b2_sb)
                m3 = mlpp.tile([D_OUT, 512], f32, tag="m3")
                nc.tensor.matmul(m3, lhsT=w3t_sb, rhs=h2,
                                 start=True, stop=True)
                h3 = hp.tile([D_OUT, 512], f32, tag="h3")
                nc.scalar.activation(h3, m3, AF.Identity, bias=b3_sb)
                for k in range(4):
                    ko = h * 4 + k
                    m4 = mlpp.tile([P, D_OUT], f32, tag="m4")
                    nc.tensor.transpose(m4, h3[:, k * P:(k + 1) * P],
                                        ident[0:D_OUT, 0:D_OUT])
                    ob = hp.tile([P, D_OUT], f32, tag="ob")
                    nc.vector.tensor_copy(ob, m4)
                    nc.sync.dma_start(out_d[ko * P:(ko + 1) * P, :], ob)

    nc.compile()
    return nc


def host_prep(x, adj, W_gat, a, gamma, beta, W1, b1, W2, b2, W3, b3,
              num_cores=N_CORES):
    bf16 = ml_dtypes.bfloat16
    n = x.shape[0]
    rows = n // num_cores
    n_chunk = n // P
    n_sc = max(1, n_chunk // SC_CHUNKS)
    sc_chunks = n_chunk // n_sc
    Wh = (x @ W_gat.T).astype(np.float32)
    s = (Wh @ a.T).astype(np.float32).ravel()
    u = np.exp(s)
    whu = np.zeros((n, D_AUG), np.float32)
    whu[:, :D_H] = Wh * u[:, None]
    whu[:, 64] = u
    # reorder [n, D_AUG] -> [P, n_chunk * D_AUG] with row p holding chunks
    whu_r = np.ascontiguousarray(
        whu.reshape(n_chunk, P, D_AUG).transpose(1, 0, 2)
        .reshape(P, n_chunk * D_AUG)).astype(bf16)
    sJm = np.ascontiguousarray((-0.8 * s).reshape(n_chunk, P).T)
    s_bf = s.astype(bf16).astype(np.float32)
    adjT = np.ascontiguousarray(adj.T)  # adjT[j, i] = adj[i, j]
    in_maps = []
    for c in range(num_cores):
        r = slice(c * rows, (c + 1) * rows)
        M = np.where(adjT[:, r] > 0, s_bf[r][None, :],
                     np.float32(MASK_VAL)).astype(bf16)
        M = np.ascontiguousarray(
            M.reshape(n_sc, sc_chunks, P, rows).transpose(0, 2, 1, 3)
            .reshape(n_sc * P, sc_chunks * rows))
        in_maps.append({
            "adjm": M,
            "whu": whu_r,
            "sJm": sJm,
            "gamma": np.ascontiguousarray(gamma[None, :]).astype(np.float32),
            "beta": np.ascontiguousarray(beta[None, :]).astype(np.float32),
            "w1t": np.ascontiguousarray(W1.T).astype(np.float32),
            "b1": np.ascontiguousarray(b1[:, None]).astype(np.float32),
            "w2t": np.ascontiguousarray(W2.T).astype(np.float32),
            "b2": np.ascontiguousarray(b2[:, None]).astype(np.float32),
            "w3t": np.ascontiguousarray(W3.T).astype(np.float32),
            "b3": np.ascontiguousarray(b3[:, None]).astype(np.float32),
        })
    return in_maps


_NC_CACHE = {}


def kernel(x, adj, W_gat, a, gamma, beta, W1, b1, W2, b2, W3, b3,
           trace=False):
    from concourse.bass_utils import run_bass_kernel_spmd

    args = [np.asarray(t) for t in
            (x, adj, W_gat, a, gamma, beta, W1, b1, W2, b2, W3, b3)]
    in_maps = host_prep(*args)
    if "nc" not in _NC_CACHE:
        _NC_CACHE["nc"] = build_nc()
    nc = _NC_CACHE["nc"]
    res = run_bass_kernel_spmd(nc, in_maps, list(range(N_CORES)), trace=trace)
    out = np.concatenate([r["out"] for r in res.results], axis=0)
    if trace:
        kernel.last_results = res
    return out.astype(np.float32)


# revision 10
# speedup vs baseline: 2.2345x; 1.4758x over previous
"""Trainium2 Bass kernel for nn_MetaRL_LightGAT_BiACT (GAT + LayerNorm + MLP).

Strategy (8 NeuronCores, row-sharded, transposed layout [j_part, i_free]):
  - Each core owns 1024 of the 8192 output rows (node dim N=i); the full
    j dim (8192) is reduced on-chip via PSUM accumulation.
  - Host precomputes the tiny GAT projection Wh = x @ W_gat.T and scores
    s = Wh @ a.T (0.15% of FLOPs), and marshals adj into a single bf16
    tensor  adjm[j, i] = adj[i, j] ? s_i : -60   (pre-transposed and
    pre-tiled so each superchunk is one contiguous [128, 8192] DMA slab).
  - Identity used on device, per element (w = adjm):
        adj * exp(prelu(s_i + s_j))
      = exp(max(w, 0.2*w - 0.8*s_j) + s_j)            (w = s_i on edges)
      = exp(max(w, 0.2*w - 0.8*s_j)) * e^{s_j}
    with e^{s_j} folded into the matmul weights WhU[j,:] = e^{s_j}*Wh[j,:]
    (and the softmax-denominator ones column becomes e^{s_j}).
    Non-edges (w = -60) give exp(<= -11) ~ 0, i.e. the mask.
  - Device main loop per superchunk (1024 j's):
      DVE  tensor_scalar (bf16): t1 = 0.2*w - 0.8*s_j     (per 128-chunk)
      DVE  tensor_tensor (bf16 2x): q = max(w, t1)        (whole slab)
      ACT  activation    Exp       : q = exp(q)           (whole slab)
      PE   matmul bf16: acc[65, i] += WhU_chunk^T @ q_chunk  (PSUM accum,
           col 64 of WhU is e^{s_j} -> softmax denominator D for free)
  - Epilogue (per 512-col half): LayerNorm is invariant to the positive
    per-column scale 1/D except through eps:
        (h'-mu)/sqrt(var+eps) = (num - mu_num) / sqrt(var_num + eps*D^2)
    so the attention normalization is never applied explicitly. The
    [65, 512] accumulator is PE-transposed to natural [i_part, d] layout
    where all per-i stats (sums, sqrt, divide) vectorize across 128 lanes,
    gamma/beta are folded into W1/b1 on host, and the 48->256->128->32 MLP
    runs in bf16 after transposing back.
"""

import sys

if "/opt/trn_rl_repo" not in sys.path:
    sys.path.insert(0, "/opt/trn_rl_repo")

import numpy as np
import ml_dtypes

N = 8192
D_IN = 128
D_H = 48
D_AUG = 65  # WhU cols 0-47, zeros 48-63, e^{s_j} col at 64
D_OUT = 32
N_CORES = 8
ROWS = N // N_CORES          # 1024 rows per core
P = 128                      # partitions
SC_CHUNKS = 8                # j-chunks per superchunk
MASK_VAL = -60.0
EPS = 1e-5


def build_nc(num_cores=N_CORES, rows=ROWS, n=N, reps=1, gp_ts=0):
    import concourse.bass as bass
    import concourse.mybir as mybir
    import concourse.tile as tile
    from concourse import bacc
    from concourse.masks import make_identity
    from contextlib import ExitStack

    f32 = mybir.dt.float32
    bf16 = mybir.dt.bfloat16
    AF = mybir.ActivationFunctionType
    OP = mybir.AluOpType
    AX = mybir.AxisListType

    n_chunk = n // P
    n_sc = max(1, n_chunk // SC_CHUNKS)
    sc_chunks = n_chunk // n_sc
    n_half = rows // 512

    nc = bacc.Bacc("TRN2", target_bir_lowering=False, debug=False,
                   num_devices=num_cores)

    adjm_d = nc.dram_tensor("adjm", [n_sc * P, sc_chunks * rows], bf16,
                            kind="ExternalInput").ap()
    whu_d = nc.dram_tensor("whu", [P, n_chunk * D_AUG], bf16,
                           kind="ExternalInput").ap()
    sJm_d = nc.dram_tensor("sJm", [P, n_chunk], f32, kind="ExternalInput").ap()
    w1g_d = nc.dram_tensor("w1g", [D_H, 256], bf16, kind="ExternalInput").ap()
    b1_d = nc.dram_tensor("b1", [256, 1], f32, kind="ExternalInput").ap()
    w2t_d = nc.dram_tensor("w2t", [256, 128], bf16, kind="ExternalInput").ap()
    b2_d = nc.dram_tensor("b2", [128, 1], f32, kind="ExternalInput").ap()
    w3t_d = nc.dram_tensor("w3t", [128, D_OUT], bf16, kind="ExternalInput").ap()
    b3_d = nc.dram_tensor("b3", [D_OUT, 1], f32, kind="ExternalInput").ap()
    out_d = nc.dram_tensor("out", [rows, D_OUT], f32, kind="ExternalOutput").ap()

    with ExitStack() as ctx:
        tc = ctx.enter_context(tile.TileContext(nc))
        singles = ctx.enter_context(tc.tile_pool(name="singles", bufs=1))
        adjp = ctx.enter_context(tc.tile_pool(name="adjp", bufs=3))
        t1p = ctx.enter_context(tc.tile_pool(name="t1p", bufs=2))
        qp = ctx.enter_context(tc.tile_pool(name="qp", bufs=2))
        hp = ctx.enter_context(tc.tile_pool(name="hp", bufs=2))

        # ---- resident small tensors (scalar HWDGE queue; keep Sync free
        #      for the big adjm streams) ----
        whu_sb = singles.tile([P, n_chunk, D_AUG], bf16)
        nc.scalar.dma_start(whu_sb, whu_d.rearrange("p (c d) -> p c d",
                                                    d=D_AUG))
        sJm_sb = singles.tile([P, n_chunk], f32)
        nc.scalar.dma_start(sJm_sb, sJm_d)
        w1g_sb = singles.tile([D_H, 256], bf16)
        nc.scalar.dma_start(w1g_sb, w1g_d)
        w2t_sb = singles.tile([P, 2, 128], bf16)
        nc.scalar.dma_start(w2t_sb, w2t_d.rearrange("(m p) k -> p m k", p=P))
        w3t_sb = singles.tile([P, D_OUT], bf16)
        nc.scalar.dma_start(w3t_sb, w3t_d)
        b1_sb = singles.tile([P, 2], f32)
        nc.scalar.dma_start(b1_sb, b1_d.rearrange("(m p) one -> p (m one)",
                                                  p=P))
        b2_sb = singles.tile([P, 1], f32)
        nc.scalar.dma_start(b2_sb, b2_d)
        b3_sb = singles.tile([D_OUT, 1], f32)
        nc.scalar.dma_start(b3_sb, b3_d)
        ident = singles.tile([P, P], f32)
        make_identity(nc, ident)
        identb = singles.tile([P, P], bf16)
        make_identity(nc, identb)

        # ---- main loop: masked attention scores + aggregation ----
        for rep in range(reps):
          accS = []
          with tc.tile_pool(name=f"accp{rep}", bufs=n_half, space="PSUM") as accp:
            acc = [accp.tile([D_AUG, 512], f32, tag="acc", name=f"acc{i}")
                   for i in range(n_half)]
            for sc in range(n_sc):
                adjm = adjp.tile([P, sc_chunks, rows], bf16)
                nc.sync.dma_start(
                    adjm.rearrange("p a b -> p (a b)"),
                    adjm_d[sc * P:(sc + 1) * P, :])
                t1 = t1p.tile([P, sc_chunks, rows], bf16)
                ts_eng = nc.gpsimd if sc < gp_ts else nc.vector
                for cc in range(sc_chunks):
                    jc = sc * sc_chunks + cc
                    ts_eng.tensor_scalar(
                        t1[:, cc, :], adjm[:, cc, :],
                        0.2, sJm_sb[:, jc:jc + 1], OP.mult, OP.add)
                q = qp.tile([P, sc_chunks, rows], bf16)
                qf = q.rearrange("p a b -> p (a b)")
                nc.vector.tensor_tensor(
                    qf, adjm.rearrange("p a b -> p (a b)"),
                    t1.rearrange("p a b -> p (a b)"), OP.max)
                nc.scalar.activation(qf, qf, AF.Exp)
                for cc in range(sc_chunks):
                    jc = sc * sc_chunks + cc
                    for h in range(n_half):
                        nc.tensor.matmul(
                            acc[h][:, :],
                            lhsT=whu_sb[:, jc, :],
                            rhs=q[:, cc, h * 512:(h + 1) * 512],
                            start=(jc == 0),
                            stop=(jc == n_chunk - 1))

            # evacuate accumulators to SBUF so PSUM banks free up
            for h in range(n_half):
                aS = hp.tile([D_AUG, 512], f32, tag="accS", bufs=n_half)
                nc.vector.tensor_copy(aS, acc[h])
                accS.append(aS)

          # ---- epilogue: transpose to natural layout, LN stats, MLP ----
          with tc.tile_pool(name=f"mlpp{rep}", bufs=1, space="PSUM") as mlpp:
            for h in range(n_half):
                accn = hp.tile([P, 4, D_AUG], f32, tag="accn", bufs=2)
                for k in range(4):
                    tp = mlpp.tile([P, D_AUG], f32, tag="tp", bufs=2)
                    nc.tensor.transpose(tp, accS[h][:, k * P:(k + 1) * P],
                                        ident[0:D_AUG, 0:D_AUG])
                    nc.vector.tensor_copy(accn[:, k, :], tp)
                num = accn[:, :, 0:D_H]                    # [128, 4, 48]
                Dn = accn[:, :, 64:65].rearrange("p a one -> p (a one)")
                ssum = hp.tile([P, 4], f32, tag="ssum")
                nc.vector.tensor_reduce(ssum, num, axis=AX.X, op=OP.add)
                sqt = hp.tile([P, 4, D_H], f32, tag="sqt")
                nc.vector.tensor_tensor(sqt, num, num, OP.mult)
                ssq = hp.tile([P, 4], f32, tag="ssq")
                nc.vector.tensor_reduce(ssq, sqt, axis=AX.X, op=OP.add)
                mu = hp.tile([P, 4], f32, tag="mu")
                nc.scalar.activation(mu, ssum, AF.Copy, scale=1.0 / D_H)
                var = hp.tile([P, 4], f32, tag="var")
                nc.scalar.activation(var, ssq, AF.Copy, scale=1.0 / D_H)
                musq = hp.tile([P, 4], f32, tag="musq")
                nc.vector.tensor_tensor(musq, mu, mu, OP.mult)
                nc.vector.tensor_tensor(var, var, musq, OP.subtract)
                dsq = hp.tile([P, 4], f32, tag="dsq")
                nc.vector.tensor_tensor(dsq, Dn, Dn, OP.mult)
                nc.vector.tensor_scalar(dsq, dsq, EPS, None, OP.mult)
                nc.vector.tensor_tensor(var, var, dsq, OP.add)
                std = hp.tile([P, 4], f32, tag="std")
                nc.scalar.activation(std, var, AF.Sqrt)
                f = hp.tile([P, 4], f32, tag="f")
                nc.vector.reciprocal(f, std)
                hn = hp.tile([P, 4, D_H], bf16, tag="hn")
                for k in range(4):
                    nc.vector.tensor_scalar(
                        hn[:, k, :], num[:, k, :],
                        mu[:, k:k + 1], f[:, k:k + 1],
                        OP.subtract, OP.mult)
                hT = hp.tile([D_H, 512], bf16, tag="hT", bufs=2)
                for k in range(4):
                    tph = mlpp.tile([D_H, P], bf16, tag="tph", bufs=2)
                    nc.tensor.transpose(tph, hn[:, k, :], identb)
                    nc.vector.tensor_copy(hT[:, k * P:(k + 1) * P], tph)
                # MLP head 48 -> 256 -> 128 -> 32 (bf16 weights)
                h1 = hp.tile([P, 2, 512], bf16, tag="h1")
                for m in range(2):
                    m1 = mlpp.tile([P, 512], f32, tag="m1")
                    nc.tensor.matmul(m1, lhsT=w1g_sb[:, m * P:(m + 1) * P],
                                     rhs=hT, start=True, stop=True)
                    nc.scalar.activation(h1[:, m, :], m1, AF.Relu,
                                         bias=b1_sb[:, m:m + 1])
                m2 = mlpp.tile([P, 512], f32, tag="m2")
                for m in range(2):
                    nc.tensor.matmul(m2, lhsT=w2t_sb[:, m, :],
                                     rhs=h1[:, m, :],
                                     start=(m == 0), stop=(m == 1))
                h2 = hp.tile([P, 512], bf16, tag="h2")
                nc.scalar.activation(h2, m2, AF.Relu, bias=b2_sb)
                m3 = mlpp.tile([D_OUT, 512], f32, tag="m3")
                nc.tensor.matmul(m3, lhsT=w3t_sb, rhs=h2,
                                 start=True, stop=True)
                h3 = hp.tile([D_OUT, 512], f32, tag="h3")
                nc.scalar.activation(h3, m3, AF.Identity, bias=b3_sb)
                for k in range(4):
                    ko = h * 4 + k
                    m4 = mlpp.tile([P, D_OUT], f32, tag="m4")
                    nc.tensor.transpose(m4, h3[:, k * P:(k + 1) * P],
                                        ident[0:D_OUT, 0:D_OUT])
                    ob = hp.tile([P, D_OUT], f32, tag="ob")
                    nc.vector.tensor_copy(ob, m4)
                    nc.sync.dma_start(out_d[ko * P:(ko + 1) * P, :], ob)

    nc.compile()
    return nc


def host_prep(x, adj, W_gat, a, gamma, beta, W1, b1, W2, b2, W3, b3,
              num_cores=N_CORES):
    bf16 = ml_dtypes.bfloat16
    n = x.shape[0]
    rows = n // num_cores
    n_chunk = n // P
    n_sc = max(1, n_chunk // SC_CHUNKS)
    sc_chunks = n_chunk // n_sc
    Wh = (x @ W_gat.T).astype(np.float32)
    s = (Wh @ a.T).astype(np.float32).ravel()
    u = np.exp(s)
    whu = np.zeros((n, D_AUG), np.float32)
    whu[:, :D_H] = Wh * u[:, None]
    whu[:, 64] = u
    whu_r = np.ascontiguousarray(
        whu.reshape(n_chunk, P, D_AUG).transpose(1, 0, 2)
        .reshape(P, n_chunk * D_AUG)).astype(bf16)
    sJm = np.ascontiguousarray((-0.8 * s).reshape(n_chunk, P).T)
    s_bf = s.astype(bf16).astype(np.float32)
    # fold LayerNorm gamma/beta into the first MLP layer
    W1g = (W1 * gamma[None, :]).astype(np.float32)
    b1g = (b1 + W1 @ beta).astype(np.float32)
    adjT = np.ascontiguousarray(adj.T)  # adjT[j, i] = adj[i, j]
    in_maps = []
    for c in range(num_cores):
        r = slice(c * rows, (c + 1) * rows)
        M = np.where(adjT[:, r] > 0, s_bf[r][None, :],
                     np.float32(MASK_VAL)).astype(bf16)
        M = np.ascontiguousarray(
            M.reshape(n_sc, sc_chunks, P, rows).transpose(0, 2, 1, 3)
            .reshape(n_sc * P, sc_chunks * rows))
        in_maps.append({
            "adjm": M,
            "whu": whu_r,
            "sJm": sJm,
            "w1g": np.ascontiguousarray(W1g.T).astype(bf16),
            "b1": np.ascontiguousarray(b1g[:, None]).astype(np.float32),
            "w2t": np.ascontiguousarray(W2.T).astype(bf16),
            "b2": np.ascontiguousarray(b2[:, None]).astype(np.float32),
            "w3t": np.ascontiguousarray(W3.T).astype(bf16),
            "b3": np.ascontiguousarray(b3[:, None]).astype(np.float32),
        })
    return in_maps


_NC_CACHE = {}


def kernel(x, adj, W_gat, a, gamma, beta, W1, b1, W2, b2, W3, b3,
           trace=False):
    from concourse.bass_utils import run_bass_kernel_spmd

    args = [np.asarray(t) for t in
            (x, adj, W_gat, a, gamma, beta, W1, b1, W2, b2, W3, b3)]
    in_maps = host_prep(*args)
    if "nc" not in _NC_CACHE:
        _NC_CACHE["nc"] = build_nc()
    nc = _NC_CACHE["nc"]
    res = run_bass_kernel_spmd(nc, in_maps, list(range(N_CORES)), trace=trace)
    out = np.concatenate([r["out"] for r in res.results], axis=0)
    if trace:
        kernel.last_results = res
    return out.astype(np.float32)
